# revision 14
# baseline (speedup 1.0000x reference)
"""Trainium2 Bass kernel for nn_AbstractLiquidRecurrent (liquid time-constant RNN).

Model (reference):
    x0 = 0
    per time step t (T=256):   inp = i_t @ W_in.T + b    [B,N]
      per unfold k (K=6):      f = tanh(x @ W_rec.T + inp)
                               x = (x + dt_k*f*A) / (1 + dt_k*(1/tau + f))
    output: all x_t stacked -> [B, T, N]

Kernel strategy (8 NeuronCores, data-parallel over batch, 16 rows/core):
  - State kept TRANSPOSED: y.T as [n (4 chunks of 128 partitions), b=16 free],
    so the recurrent matmul is W-stationary / x-moving and never needs an
    in-loop transpose.
  - A is folded into the weights host-side (Wt = diag(A) @ W_rec.T, state
    y = x/A), which simplifies the elementwise update to
        y' = (y*R + f) / (R + 1/tau + f),   R = K/dt   (per batch, per t)
  - Matmul dtype modes:
      "f32"    exact fp32 matmuls (432 ns/tile on HW, weight-load bound)
      "bf16"   single bf16 pass (39 ns/tile); moving operand read as the
               high 16 bits of the fp32 state via a strided AP (free cast)
      "split6" 3-way bf16 decomposition of both W and y; the 6 dominant
               cross products give ~24-bit effective mantissas (fp32-grade)
               at 6x39=234 ns/tile -- ~2x faster than "f32"
  - Input projection accumulated into a PSUM bank once per t; each unfold's
    matmul PSUM banks are PRELOADED with it via a ScalarE copy (has_written
    bits armed once at kernel start, so start=False matmuls accumulate).
  - tanh on ScalarE; reciprocal via custom DVE Newton ops (tanh and
    reciprocal cannot share an ACT table set).
  - 2-group software pipeline over the 4 n-chunks; the next unfold's
    matmuls are ordered to consume group-0 state columns first.
  - Output y_t is DMA'd out per t in transposed layout; the host unshards,
    transposes back, and multiplies by A.
"""

import time as _time

import numpy as np

import concourse.bass as bass
import concourse.tile as tile
from concourse import bacc, mybir
from concourse.bass_utils import run_bass_kernel_spmd

# Problem constants (hardcoded per contract)
N = 512
F = 256
KUNF = 6
B, T = 128, 256
NCORES = 8
BLOC = B // NCORES          # 16 batch rows per core
NCH = N // 128              # 4 n-chunks
FCH = F // 128              # 2 f-chunks

f32 = mybir.dt.float32
bf16 = mybir.dt.bfloat16

MM_DTYPE = "split6"
RECIP = "accurate"          # "fast" (1 op, 51 ULP) | "accurate" (2 ops, 2 ULP)
VERBOSE = True


def _bf16_split(arr, terms):
    """Split fp32 array into `terms` bf16 arrays summing to ~arr."""
    import ml_dtypes
    out = []
    rem = np.asarray(arr, dtype=np.float32).copy()
    for _ in range(terms):
        h = rem.astype(ml_dtypes.bfloat16)
        out.append(np.ascontiguousarray(h))
        rem = rem - h.astype(np.float32)
    return out


def _hi_view(ap):
    """bf16 view of the high 16 bits of an f32 AP (truncated bf16 cast)."""
    p, n = ap.shape
    return ap.bitcast(bf16).rearrange("p (n two) -> p n two", two=2)[:, :, 1]


def build(t_run=T, mm_mode=MM_DTYPE):
    """Build + compile the Bass module for one core (SPMD across 8)."""
    t0 = _time.time()
    nc = bacc.Bacc("TRN2", target_bir_lowering=False, debug=False,
                   disable_frame_to_traceback=True)

    n_w = {"f32": 1, "bf16": 1, "split6": 3}[mm_mode]
    mdt = f32 if mm_mode == "f32" else bf16

    # ---- DRAM I/O ----
    w_d = [nc.dram_tensor(f"wrec{j}", [128, NCH * NCH * 128], mdt,
                          kind="ExternalInput").ap() for j in range(n_w)]
    win_d = [nc.dram_tensor(f"win{j}", [128, FCH * NCH * 128], mdt,
                            kind="ExternalInput").ap() for j in range(n_w)]
    it_d = [nc.dram_tensor(f"it{j}", [128, t_run * FCH * BLOC], mdt,
                           kind="ExternalInput").ap() for j in range(n_w)]
    rt_d = nc.dram_tensor("rt", [1, t_run * BLOC], f32, kind="ExternalInput").ap()
    invtau_d = nc.dram_tensor("invtau", [128, NCH], f32, kind="ExternalInput").ap()
    bvec_d = nc.dram_tensor("bvec", [128, NCH], f32, kind="ExternalInput").ap()
    yout_d = nc.dram_tensor("yout", [t_run, 128, NCH * BLOC], f32,
                            kind="ExternalOutput").ap()

    W = NCH * BLOC   # 64 free width of merged state tiles
    G = 2            # pipeline groups (2 n-chunks each)
    GW = W // G      # 32 free width per group

    with tile.TileContext(nc) as tc:
        import contextlib
        ctx = contextlib.ExitStack()
        with ctx:
            consts = ctx.enter_context(tc.tile_pool(name="consts", bufs=1))
            state = ctx.enter_context(tc.tile_pool(name="state", bufs=3))
            work = ctx.enter_context(tc.tile_pool(name="work", bufs=2))
            prep = ctx.enter_context(tc.tile_pool(name="prep", bufs=2))
            psum = ctx.enter_context(tc.tile_pool(name="psum", bufs=1, space="PSUM"))

            # ---- constant loads ----
            w_sb, win_sb, it_sb = [], [], []
            for j in range(n_w):
                wj = consts.tile([128, NCH * NCH * 128], mdt, name=f"w_sb{j}")
                nc.sync.dma_start(wj[:], w_d[j][:])
                w_sb.append(wj)
                winj = consts.tile([128, FCH * NCH * 128], mdt, name=f"win_sb{j}")
                nc.sync.dma_start(winj[:], win_d[j][:])
                win_sb.append(winj)
                itj = consts.tile([128, t_run * FCH * BLOC], mdt, name=f"it_sb{j}")
                nc.sync.dma_start(itj[:], it_d[j][:])
                it_sb.append(itj)
            rt_sb = consts.tile([1, t_run * BLOC], f32)
            nc.sync.dma_start(rt_sb[:], rt_d[:])
            invtau_sb = consts.tile([128, NCH], f32)
            nc.sync.dma_start(invtau_sb[:], invtau_d[:])
            bvec_sb = consts.tile([128, NCH], f32)
            nc.sync.dma_start(bvec_sb[:], bvec_d[:])
            ones_sb = consts.tile([1, 128], f32)
            nc.vector.memset(ones_sb[:], 1.0)
            junk1 = consts.tile([1, GW], mdt)
            nc.vector.memset(junk1[:], 0.0)
            junk2 = consts.tile([1, 128], mdt)
            nc.vector.memset(junk2[:], 0.0)

            # persistent PSUM tiles
            zg = [psum.tile([128, GW], f32, name=f"zg{g}", tag=f"zg{g}")
                  for g in range(G)]
            pin = psum.tile([128, W], f32, tag="pin")
            prt = psum.tile([128, BLOC], f32, tag="prt")

            # arm has_written bits of the z banks once
            for g in range(G):
                nc.tensor.matmul(zg[g][:], lhsT=junk2[:], rhs=junk1[:],
                                 start=True, stop=True)

            # initial state y = 0 (+ split terms), yR = 0.
            # Every loop-carried / epilogue tile is PER GROUP: Tile tracks
            # dependencies at whole-tile granularity, so sharing a [128, W]
            # tile between the two pipeline groups would serialize them.
            y_cur, yr_cur, y1_cur, y2_cur = [], [], [], []
            for g in range(G):
                yg = state.tile([128, GW], f32, name=f"y_init{g}", tag=f"y{g}")
                nc.vector.memset(yg[:], 0.0)
                y_cur.append(yg)
                yrg = state.tile([128, GW], f32, name=f"yr_init{g}", tag=f"yr{g}")
                nc.vector.memset(yrg[:], 0.0)
                yr_cur.append(yrg)
                if mm_mode == "split6":
                    y1g = state.tile([128, GW], bf16, name=f"y1_init{g}", tag=f"y1{g}")
                    nc.vector.memset(y1g[:], 0.0)
                    y1_cur.append(y1g)
                    y2g = state.tile([128, GW], bf16, name=f"y2_init{g}", tag=f"y2{g}")
                    nc.vector.memset(y2g[:], 0.0)
                    y2_cur.append(y2g)

            def w_tile(j, kc, mc):
                off = (kc * NCH + mc) * 128
                return w_sb[j][:, off:off + 128]

            def win_tile(j, fc, mc):
                off = (fc * NCH + mc) * 128
                return win_sb[j][:, off:off + 128]

            # matmul pass list: (w_index, y_source_index) ordered so that
            # later passes depend on later-computed y split terms.
            if mm_mode == "split6":
                mm_passes = [(0, 0), (1, 0), (0, 1), (2, 0), (1, 1), (0, 2)]
            else:
                mm_passes = [(0, 0)]

            def y_sources():
                """Moving-operand sources, indexed [split][group]."""
                if mm_mode == "f32":
                    return [[y_cur[g][:] for g in range(G)]]
                if mm_mode == "bf16":
                    return [[_hi_view(y_cur[g][:]) for g in range(G)]]
                return [[_hi_view(y_cur[g][:]) for g in range(G)],
                        [y1_cur[g][:] for g in range(G)],
                        [y2_cur[g][:] for g in range(G)]]

            def prep_t(t):
                """Per-time-step prep: input projection, R tile, P2, inp+b."""
                # mc outer so each PSUM region's accumulation group is
                # contiguous (start=True clears has_written for the whole
                # bank, so a region must not be revisited after a later
                # start=True).
                seq = [(pj, sj, fc) for (pj, sj) in mm_passes
                       for fc in range(FCH)]
                for mc in range(NCH):
                    for idx, (pj, sj, fc) in enumerate(seq):
                        nc.tensor.matmul(
                            pin[:, mc * BLOC:(mc + 1) * BLOC],
                            lhsT=win_tile(pj, fc, mc),
                            rhs=it_sb[sj][:, (t * FCH + fc) * BLOC:
                                          (t * FCH + fc + 1) * BLOC],
                            start=(mc == 0 and idx == 0),
                            stop=(idx == len(seq) - 1),
                            skip_group_check=True,
                        )
                nc.tensor.matmul(prt[:], lhsT=ones_sb[:],
                                 rhs=rt_sb[:, t * BLOC:(t + 1) * BLOC],
                                 start=True, stop=True)
                rtile = prep.tile([128, BLOC], f32, tag="rtile")
                nc.scalar.activation(rtile[:], prt[:],
                                     mybir.ActivationFunctionType.Copy)
                p2 = prep.tile([128, W], f32, tag="p2")
                nc.vector.tensor_add(
                    p2[:],
                    rtile[:].unsqueeze(1).broadcast_to([128, NCH, BLOC]),
                    invtau_sb[:].unsqueeze(2).broadcast_to([128, NCH, BLOC]),
                )
                inp = prep.tile([128, W], f32, tag="inp")
                nc.vector.tensor_add(
                    inp[:],
                    pin[:],
                    bvec_sb[:].unsqueeze(2).broadcast_to([128, NCH, BLOC]),
                )
                return rtile, p2, inp

            rtile, p2, inp = prep_t(0)
            for g in range(G):
                nc.scalar.activation(zg[g][:], inp[:, g * GW:(g + 1) * GW],
                                     mybir.ActivationFunctionType.Copy)

            for t in range(t_run):
                nxt = None
                for k in range(KUNF):
                    last_unfold = (k == KUNF - 1)
                    ys = y_sources()
                    y_new = [None] * G
                    yr_new = [None] * G
                    y1_new = [None] * G
                    y2_new = [None] * G

                    # bank-major: all of bank g's matmuls, then its epilogue
                    # (overlapping the other bank's matmuls on the PE).
                    for g in range(G):
                        mcs = range(g * (NCH // G), (g + 1) * (NCH // G))
                        cnt = 0
                        total = len(mm_passes) * (NCH // G) * NCH
                        for kcp in ((0, 1), (2, 3)):
                            for pj, sj in mm_passes:
                                for mc in mcs:
                                    sub = mc % (NCH // G)
                                    for kc in kcp:
                                        cnt += 1
                                        src_g, src_c = divmod(kc, NCH // G)
                                        nc.tensor.matmul(
                                            zg[g][:, sub * BLOC:(sub + 1) * BLOC],
                                            lhsT=w_tile(pj, kc, mc),
                                            rhs=ys[sj][src_g][:, src_c * BLOC:
                                                              (src_c + 1) * BLOC],
                                            start=False, stop=(cnt == total),
                                            skip_group_check=True,
                                        )
                        # ---- epilogue for bank g (per-group tiles) ----
                        f_t = work.tile([128, GW], f32, name=f"f{g}", tag=f"f{g}")
                        d_t = work.tile([128, GW], f32, name=f"d{g}", tag=f"d{g}")
                        rden_t = work.tile([128, GW], f32, name=f"rden{g}", tag=f"rden{g}")
                        nm_t = work.tile([128, GW], f32, name=f"nm{g}", tag=f"nm{g}")
                        rscr_t = work.tile([128, GW], f32, name=f"rscr{g}", tag=f"rscr{g}")
                        trem_t = work.tile([128, GW], f32, name=f"trem{g}", tag=f"trem{g}")
                        yg = state.tile([128, GW], f32, name=f"y_n{g}", tag=f"y{g}")
                        y_new[g] = yg
                        sl = slice(g * GW, (g + 1) * GW)
                        nc.scalar.activation(f_t[:], zg[g][:],
                                             mybir.ActivationFunctionType.Tanh)
                        src = inp if (not last_unfold or t + 1 >= t_run) else nxt[2]
                        nc.scalar.activation(zg[g][:], src[:, sl],
                                             mybir.ActivationFunctionType.Copy)
                        nc.vector.tensor_add(d_t[:], f_t[:], p2[:, sl])
                        nc.vector.tensor_add(nm_t[:], f_t[:], yr_cur[g][:])
                        if RECIP == "accurate":
                            nc.vector.reciprocal_approx_accurate(
                                out=rden_t[:], in_=d_t[:], scratch=rscr_t[:])
                        else:
                            nc.vector.reciprocal_approx_fast(out=rden_t[:],
                                                             in_=d_t[:])
                        nc.vector.tensor_mul(yg[:], nm_t[:], rden_t[:])
                        if mm_mode == "split6":
                            y1g = state.tile([128, GW], bf16, name=f"y1_n{g}", tag=f"y1{g}")
                            y2g = state.tile([128, GW], bf16, name=f"y2_n{g}", tag=f"y2{g}")
                            y1_new[g], y2_new[g] = y1g, y2g
                            nc.vector.tensor_sub(trem_t[:], yg[:], _hi_view(yg[:]))
                            nc.vector.tensor_copy(y1g[:], trem_t[:])
                            nc.vector.tensor_sub(y2g[:], trem_t[:], y1g[:])
                        if not last_unfold:
                            yrg = state.tile([128, GW], f32, name=f"yr_n{g}", tag=f"yr{g}")
                            yr_new[g] = yrg
                            nc.vector.tensor_mul(
                                yrg[:], yg[:],
                                rtile[:].unsqueeze(1).broadcast_to(
                                    [128, NCH // G, BLOC]),
                            )
                        # mid-unfold prep for t+1 after bank 0 of unfold 2
                        if g == 0 and k == 2 and t + 1 < t_run:
                            nxt = prep_t(t + 1)
                    y_cur = y_new
                    if mm_mode == "split6":
                        y1_cur, y2_cur = y1_new, y2_new
                    if not last_unfold:
                        yr_cur = yr_new

                # ---- end of time step: output + roll prep to t+1 ----
                for g in range(G):
                    nc.sync.dma_start(yout_d[t][:, g * GW:(g + 1) * GW],
                                      y_cur[g][:])
                if t + 1 < t_run:
                    rtile, p2, inp = nxt
                    yr_cur = []
                    for g in range(G):
                        yrg = state.tile([128, GW], f32, name=f"yr_roll{g}",
                                         tag=f"yr{g}")
                        nc.vector.tensor_mul(
                            yrg[:], y_cur[g][:],
                            rtile[:].unsqueeze(1).broadcast_to(
                                [128, NCH // G, BLOC]),
                        )
                        yr_cur.append(yrg)

    t1 = _time.time()
    nc.compile()
    t2 = _time.time()
    if VERBOSE:
        print(f"[build] trace+schedule {t1-t0:.1f}s, bacc compile {t2-t1:.1f}s",
              flush=True)
    return nc


def _host_prep(i, delta_t, W_rec, W_in, b, A, tau, t_run, mm_mode):
    """Shard + lay out the inputs for each core."""
    i = np.asarray(i, dtype=np.float32)
    delta_t = np.asarray(delta_t, dtype=np.float32)
    W_rec = np.asarray(W_rec, dtype=np.float32)
    W_in = np.asarray(W_in, dtype=np.float32)
    b = np.asarray(b, dtype=np.float32)
    A = np.asarray(A, dtype=np.float32)
    tau = np.asarray(tau, dtype=np.float32)

    n_w = {"f32": 1, "bf16": 1, "split6": 3}[mm_mode]

    def tiles_rec(m):   # [512,512] (k, m) -> [128, 16*128]
        return m.reshape(NCH, 128, NCH, 128).transpose(1, 0, 2, 3).reshape(128, -1)

    def tiles_in(m):    # [256,512] (k, m) -> [128, 8*128]
        return m.reshape(FCH, 128, NCH, 128).transpose(1, 0, 2, 3).reshape(128, -1)

    Wt = (W_rec * A[None, :]).T          # Wt[k, m] = A[k] * W_rec[m, k]
    WinT = W_in.T
    if mm_mode == "f32":
        w_arrs = [np.ascontiguousarray(tiles_rec(Wt), dtype=np.float32)]
        win_arrs = [np.ascontiguousarray(tiles_in(WinT), dtype=np.float32)]
    else:
        w_arrs = [tiles_rec(x.astype(np.float32)).astype(x.dtype)
                  for x in _bf16_split(Wt, n_w)]
        win_arrs = [tiles_in(x.astype(np.float32)).astype(x.dtype)
                    for x in _bf16_split(WinT, n_w)]
        w_arrs = [np.ascontiguousarray(x) for x in w_arrs]
        win_arrs = [np.ascontiguousarray(x) for x in win_arrs]

    invtau = np.ascontiguousarray((1.0 / tau).reshape(NCH, 128).T, dtype=np.float32)
    bvec = np.ascontiguousarray(b.reshape(NCH, 128).T, dtype=np.float32)

    in_maps = []
    for c in range(NCORES):
        bsl = slice(c * BLOC, (c + 1) * BLOC)
        ii = i[bsl, :t_run]                    # [16, t, 256]
        def tile_i(x):
            return (x.reshape(BLOC, t_run, FCH, 128)
                    .transpose(3, 1, 2, 0).reshape(128, -1))
        if mm_mode == "f32":
            it_arrs = [np.ascontiguousarray(tile_i(ii), dtype=np.float32)]
        else:
            it_arrs = [np.ascontiguousarray(tile_i(x.astype(np.float32)).astype(x.dtype))
                       for x in _bf16_split(ii, n_w)]
        rt = np.ascontiguousarray(
            (KUNF / np.maximum(delta_t[bsl, :t_run], 1e-30)).T.reshape(1, -1),
            dtype=np.float32)
        m = {"rt": rt, "invtau": invtau, "bvec": bvec}
        for j in range(n_w):
            m[f"wrec{j}"] = w_arrs[j]
            m[f"win{j}"] = win_arrs[j]
            m[f"it{j}"] = it_arrs[j]
        in_maps.append(m)
    return in_maps


def _host_unshard(results, A, t_run):
    """results[c]["yout"]: [t, 128, 64] -> full x [B, T, N] (x = A*y)."""
    A = np.asarray(A, dtype=np.float32)
    out = np.empty((B, t_run, N), dtype=np.float32)
    for c in range(NCORES):
        y = results[c]["yout"].reshape(t_run, 128, NCH, BLOC)
        xc = y.transpose(3, 0, 2, 1).reshape(BLOC, t_run, N)
        out[c * BLOC:(c + 1) * BLOC] = xc * A[None, None, :]
    return out


_BUILD_CACHE = {}


def _get_built(t_run, mm_mode):
    key = (t_run, mm_mode)
    if key not in _BUILD_CACHE:
        _BUILD_CACHE[key] = build(t_run, mm_mode)
    return _BUILD_CACHE[key]


def run(i, delta_t, W_rec, W_in, b, A, tau, t_run=T, mm_mode=MM_DTYPE, **rb_kwargs):
    nc = _get_built(t_run, mm_mode)
    in_maps = _host_prep(i, delta_t, W_rec, W_in, b, A, tau, t_run, mm_mode)
    res = run_bass_kernel_spmd(nc, in_maps, list(range(NCORES)), **rb_kwargs)
    out = _host_unshard(res.results, A, t_run)
    return out, res


def kernel(i, delta_t, W_rec, W_in, b, A, tau):
    out, _ = run(i, delta_t, W_rec, W_in, b, A, tau)
    return out


# revision 16
# speedup vs baseline: 1.0562x; 1.0562x over previous
"""Trainium2 Bass kernel for nn_AbstractLiquidRecurrent (liquid time-constant RNN).

Model (reference):
    x0 = 0
    per time step t (T=256):   inp = i_t @ W_in.T + b    [B,N]
      per unfold k (K=6):      f = tanh(x @ W_rec.T + inp)
                               x = (x + dt_k*f*A) / (1 + dt_k*(1/tau + f))
    output: all x_t stacked -> [B, T, N]

Kernel strategy (8 NeuronCores, data-parallel over batch, 16 rows/core):
  - State kept TRANSPOSED: y.T as [n (4 chunks of 128 partitions), b=16 free],
    so the recurrent matmul is W-stationary / x-moving and never needs an
    in-loop transpose.
  - A is folded into the weights host-side (Wt = diag(A) @ W_rec.T, state
    y = x/A), which simplifies the elementwise update to
        y' = (y*R + f) / (R + 1/tau + f),   R = K/dt   (per batch, per t)
  - Matmul dtype modes:
      "f32"    exact fp32 matmuls (432 ns/tile on HW, weight-load bound)
      "bf16"   single bf16 pass (39 ns/tile); moving operand read as the
               high 16 bits of the fp32 state via a strided AP (free cast)
      "split6" 3-way bf16 decomposition of both W and y; the 6 dominant
               cross products give ~24-bit effective mantissas (fp32-grade)
               at 6x39=234 ns/tile -- ~2x faster than "f32"
  - Input projection accumulated into a PSUM bank once per t; each unfold's
    matmul PSUM banks are PRELOADED with it via a ScalarE copy (has_written
    bits armed once at kernel start, so start=False matmuls accumulate).
  - tanh on ScalarE; reciprocal via custom DVE Newton ops (tanh and
    reciprocal cannot share an ACT table set).
  - 2-group software pipeline over the 4 n-chunks; the next unfold's
    matmuls are ordered to consume group-0 state columns first.
  - Output y_t is DMA'd out per t in transposed layout; the host unshards,
    transposes back, and multiplies by A.
"""

import time as _time

import numpy as np

import concourse.bass as bass
import concourse.tile as tile
from concourse.tile import add_dep_helper
from concourse import bacc, mybir
from concourse.bass_utils import run_bass_kernel_spmd

# Problem constants (hardcoded per contract)
N = 512
F = 256
KUNF = 6
B, T = 128, 256
NCORES = 8
BLOC = B // NCORES          # 16 batch rows per core
NCH = N // 128              # 4 n-chunks
FCH = F // 128              # 2 f-chunks

f32 = mybir.dt.float32
bf16 = mybir.dt.bfloat16

MM_DTYPE = "split6"
RECIP = "accurate"          # "fast" (1 op, 51 ULP) | "accurate" (2 ops, 2 ULP)
VERBOSE = True


def _bf16_split(arr, terms):
    """Split fp32 array into `terms` bf16 arrays summing to ~arr."""
    import ml_dtypes
    out = []
    rem = np.asarray(arr, dtype=np.float32).copy()
    for _ in range(terms):
        h = rem.astype(ml_dtypes.bfloat16)
        out.append(np.ascontiguousarray(h))
        rem = rem - h.astype(np.float32)
    return out


def _hi_view(ap):
    """bf16 view of the high 16 bits of an f32 AP (truncated bf16 cast)."""
    p, n = ap.shape
    return ap.bitcast(bf16).rearrange("p (n two) -> p n two", two=2)[:, :, 1]


def build(t_run=T, mm_mode=MM_DTYPE):
    """Build + compile the Bass module for one core (SPMD across 8)."""
    t0 = _time.time()
    nc = bacc.Bacc("TRN2", target_bir_lowering=False, debug=False,
                   disable_frame_to_traceback=True)

    n_w = {"f32": 1, "bf16": 1, "split6": 3}[mm_mode]
    mdt = f32 if mm_mode == "f32" else bf16

    # ---- DRAM I/O ----
    w_d = [nc.dram_tensor(f"wrec{j}", [128, NCH * NCH * 128], mdt,
                          kind="ExternalInput").ap() for j in range(n_w)]
    win_d = [nc.dram_tensor(f"win{j}", [128, FCH * NCH * 128], mdt,
                            kind="ExternalInput").ap() for j in range(n_w)]
    it_d = [nc.dram_tensor(f"it{j}", [128, t_run * FCH * BLOC], mdt,
                           kind="ExternalInput").ap() for j in range(n_w)]
    rt_d = nc.dram_tensor("rt", [1, t_run * BLOC], f32, kind="ExternalInput").ap()
    invtau_d = nc.dram_tensor("invtau", [128, NCH], f32, kind="ExternalInput").ap()
    bvec_d = nc.dram_tensor("bvec", [128, NCH], f32, kind="ExternalInput").ap()
    yout_d = nc.dram_tensor("yout", [t_run, 128, NCH * BLOC], f32,
                            kind="ExternalOutput").ap()

    W = NCH * BLOC   # 64 free width of merged state tiles
    G = 2            # pipeline groups (2 n-chunks each)
    GW = W // G      # 32 free width per group

    with tile.TileContext(nc) as tc:
        import contextlib
        ctx = contextlib.ExitStack()
        with ctx:
            consts = ctx.enter_context(tc.tile_pool(name="consts", bufs=1))
            state = ctx.enter_context(tc.tile_pool(name="state", bufs=3))
            work = ctx.enter_context(tc.tile_pool(name="work", bufs=2))
            prep = ctx.enter_context(tc.tile_pool(name="prep", bufs=2))
            psum = ctx.enter_context(tc.tile_pool(name="psum", bufs=1, space="PSUM"))

            # ---- constant loads ----
            w_sb, win_sb, it_sb = [], [], []
            for j in range(n_w):
                wj = consts.tile([128, NCH * NCH * 128], mdt, name=f"w_sb{j}")
                nc.sync.dma_start(wj[:], w_d[j][:])
                w_sb.append(wj)
                winj = consts.tile([128, FCH * NCH * 128], mdt, name=f"win_sb{j}")
                nc.sync.dma_start(winj[:], win_d[j][:])
                win_sb.append(winj)
                itj = consts.tile([128, t_run * FCH * BLOC], mdt, name=f"it_sb{j}")
                nc.sync.dma_start(itj[:], it_d[j][:])
                it_sb.append(itj)
            rt_sb = consts.tile([1, t_run * BLOC], f32)
            nc.sync.dma_start(rt_sb[:], rt_d[:])
            invtau_sb = consts.tile([128, NCH], f32)
            nc.sync.dma_start(invtau_sb[:], invtau_d[:])
            bvec_sb = consts.tile([128, NCH], f32)
            nc.sync.dma_start(bvec_sb[:], bvec_d[:])
            ones_sb = consts.tile([1, 128], f32)
            nc.vector.memset(ones_sb[:], 1.0)
            junk1 = consts.tile([1, GW], mdt)
            nc.vector.memset(junk1[:], 0.0)
            junk2 = consts.tile([1, 128], mdt)
            nc.vector.memset(junk2[:], 0.0)

            # persistent PSUM tiles
            zg = [psum.tile([128, GW], f32, name=f"zg{g}", tag=f"zg{g}")
                  for g in range(G)]
            pin = psum.tile([128, W], f32, tag="pin")
            prt = psum.tile([128, BLOC], f32, tag="prt")

            # arm has_written bits of the z banks once
            for g in range(G):
                nc.tensor.matmul(zg[g][:], lhsT=junk2[:], rhs=junk1[:],
                                 start=True, stop=True)

            # initial state y = 0 (+ split terms), yR = 0.
            # Every loop-carried / epilogue tile is PER GROUP: Tile tracks
            # dependencies at whole-tile granularity, so sharing a [128, W]
            # tile between the two pipeline groups would serialize them.
            y_cur, yr_cur, y1_cur, y2_cur = [], [], [], []
            for g in range(G):
                yg = state.tile([128, GW], f32, name=f"y_init{g}", tag=f"y{g}")
                nc.vector.memset(yg[:], 0.0)
                y_cur.append(yg)
                yrg = state.tile([128, GW], f32, name=f"yr_init{g}", tag=f"yr{g}")
                nc.vector.memset(yrg[:], 0.0)
                yr_cur.append(yrg)
                if mm_mode == "split6":
                    y1g = state.tile([128, GW], bf16, name=f"y1_init{g}", tag=f"y1{g}")
                    nc.vector.memset(y1g[:], 0.0)
                    y1_cur.append(y1g)
                    y2g = state.tile([128, GW], bf16, name=f"y2_init{g}", tag=f"y2{g}")
                    nc.vector.memset(y2g[:], 0.0)
                    y2_cur.append(y2g)

            def w_tile(j, kc, mc):
                off = (kc * NCH + mc) * 128
                return w_sb[j][:, off:off + 128]

            def win_tile(j, fc, mc):
                off = (fc * NCH + mc) * 128
                return win_sb[j][:, off:off + 128]

            # matmul pass list: (w_index, y_source_index) ordered so that
            # later passes depend on later-computed y split terms.
            if mm_mode == "split6":
                mm_passes = [(0, 0), (1, 0), (0, 1), (2, 0), (1, 1), (0, 2)]
            else:
                mm_passes = [(0, 0)]

            def y_sources():
                """Moving-operand sources, indexed [split][group]."""
                if mm_mode == "f32":
                    return [[y_cur[g][:] for g in range(G)]]
                if mm_mode == "bf16":
                    return [[_hi_view(y_cur[g][:]) for g in range(G)]]
                return [[_hi_view(y_cur[g][:]) for g in range(G)],
                        [y1_cur[g][:] for g in range(G)],
                        [y2_cur[g][:] for g in range(G)]]

            def prep_t(t):
                """Per-time-step prep: input projection, R tile, P2, inp+b."""
                # mc outer so each PSUM region's accumulation group is
                # contiguous (start=True clears has_written for the whole
                # bank, so a region must not be revisited after a later
                # start=True).
                seq = [(pj, sj, fc) for (pj, sj) in mm_passes
                       for fc in range(FCH)]
                for mc in range(NCH):
                    for idx, (pj, sj, fc) in enumerate(seq):
                        nc.tensor.matmul(
                            pin[:, mc * BLOC:(mc + 1) * BLOC],
                            lhsT=win_tile(pj, fc, mc),
                            rhs=it_sb[sj][:, (t * FCH + fc) * BLOC:
                                          (t * FCH + fc + 1) * BLOC],
                            start=(mc == 0 and idx == 0),
                            stop=(idx == len(seq) - 1),
                            skip_group_check=True,
                        )
                nc.tensor.matmul(prt[:], lhsT=ones_sb[:],
                                 rhs=rt_sb[:, t * BLOC:(t + 1) * BLOC],
                                 start=True, stop=True)
                rtile = prep.tile([128, BLOC], f32, tag="rtile")
                nc.scalar.activation(rtile[:], prt[:],
                                     mybir.ActivationFunctionType.Copy)
                p2 = prep.tile([128, W], f32, tag="p2")
                nc.vector.tensor_add(
                    p2[:],
                    rtile[:].unsqueeze(1).broadcast_to([128, NCH, BLOC]),
                    invtau_sb[:].unsqueeze(2).broadcast_to([128, NCH, BLOC]),
                )
                inp = prep.tile([128, W], f32, tag="inp")
                nc.vector.tensor_add(
                    inp[:],
                    pin[:],
                    bvec_sb[:].unsqueeze(2).broadcast_to([128, NCH, BLOC]),
                )
                return rtile, p2, inp

            rtile, p2, inp = prep_t(0)
            for g in range(G):
                nc.scalar.activation(zg[g][:], inp[:, g * GW:(g + 1) * GW],
                                     mybir.ActivationFunctionType.Copy)

            prev_last_mm = None
            for t in range(t_run):
                nxt = None
                for k in range(KUNF):
                    last_unfold = (k == KUNF - 1)
                    ys = y_sources()
                    y_new = [None] * G
                    yr_new = [None] * G
                    y1_new = [None] * G
                    y2_new = [None] * G

                    # bank-major: all of bank g's matmuls, then its epilogue
                    # (overlapping the other bank's matmuls on the PE).
                    for g in range(G):
                        mcs = range(g * (NCH // G), (g + 1) * (NCH // G))
                        cnt = 0
                        total = len(mm_passes) * (NCH // G) * NCH
                        first_mm = None
                        last_mm = None
                        for kcp in ((0, 1), (2, 3)):
                            for pj, sj in mm_passes:
                                for mc in mcs:
                                    sub = mc % (NCH // G)
                                    for kc in kcp:
                                        cnt += 1
                                        src_g, src_c = divmod(kc, NCH // G)
                                        mm = nc.tensor.matmul(
                                            zg[g][:, sub * BLOC:(sub + 1) * BLOC],
                                            lhsT=w_tile(pj, kc, mc),
                                            rhs=ys[sj][src_g][:, src_c * BLOC:
                                                              (src_c + 1) * BLOC],
                                            start=False, stop=(cnt == total),
                                            skip_group_check=True,
                                        )
                                        if first_mm is None:
                                            first_mm = mm
                                        last_mm = mm
                        # pin the scheduler to bank-block order on the PE
                        if prev_last_mm is not None:
                            add_dep_helper(first_mm.ins, prev_last_mm.ins,
                                           sync=False,
                                           reason="bank-block PE ordering")
                        prev_last_mm = last_mm
                        # ---- epilogue for bank g (per-group tiles) ----
                        f_t = work.tile([128, GW], f32, name=f"f{g}", tag=f"f{g}")
                        d_t = work.tile([128, GW], f32, name=f"d{g}", tag=f"d{g}")
                        rden_t = work.tile([128, GW], f32, name=f"rden{g}", tag=f"rden{g}")
                        nm_t = work.tile([128, GW], f32, name=f"nm{g}", tag=f"nm{g}")
                        rscr_t = work.tile([128, GW], f32, name=f"rscr{g}", tag=f"rscr{g}")
                        trem_t = work.tile([128, GW], f32, name=f"trem{g}", tag=f"trem{g}")
                        yg = state.tile([128, GW], f32, name=f"y_n{g}", tag=f"y{g}")
                        y_new[g] = yg
                        sl = slice(g * GW, (g + 1) * GW)
                        nc.scalar.activation(f_t[:], zg[g][:],
                                             mybir.ActivationFunctionType.Tanh)
                        src = inp if (not last_unfold or t + 1 >= t_run) else nxt[2]
                        nc.scalar.activation(zg[g][:], src[:, sl],
                                             mybir.ActivationFunctionType.Copy)
                        nc.vector.tensor_add(d_t[:], f_t[:], p2[:, sl])
                        nc.vector.tensor_add(nm_t[:], f_t[:], yr_cur[g][:])
                        if RECIP == "accurate":
                            nc.vector.reciprocal_approx_accurate(
                                out=rden_t[:], in_=d_t[:], scratch=rscr_t[:])
                        else:
                            nc.vector.reciprocal_approx_fast(out=rden_t[:],
                                                             in_=d_t[:])
                        nc.vector.tensor_mul(yg[:], nm_t[:], rden_t[:])
                        if mm_mode == "split6":
                            y1g = state.tile([128, GW], bf16, name=f"y1_n{g}", tag=f"y1{g}")
                            y2g = state.tile([128, GW], bf16, name=f"y2_n{g}", tag=f"y2{g}")
                            y1_new[g], y2_new[g] = y1g, y2g
                            nc.vector.tensor_sub(trem_t[:], yg[:], _hi_view(yg[:]))
                            nc.vector.tensor_copy(y1g[:], trem_t[:])
                            nc.vector.tensor_sub(y2g[:], trem_t[:], y1g[:])
                        if not last_unfold:
                            yrg = state.tile([128, GW], f32, name=f"yr_n{g}", tag=f"yr{g}")
                            yr_new[g] = yrg
                            nc.vector.tensor_mul(
                                yrg[:], yg[:],
                                rtile[:].unsqueeze(1).broadcast_to(
                                    [128, NCH // G, BLOC]),
                            )
                        # mid-unfold prep for t+1 after bank 0 of unfold 2
                        if g == 0 and k == 2 and t + 1 < t_run:
                            nxt = prep_t(t + 1)
                    y_cur = y_new
                    if mm_mode == "split6":
                        y1_cur, y2_cur = y1_new, y2_new
                    if not last_unfold:
                        yr_cur = yr_new

                # ---- end of time step: output + roll prep to t+1 ----
                for g in range(G):
                    nc.sync.dma_start(yout_d[t][:, g * GW:(g + 1) * GW],
                                      y_cur[g][:])
                if t + 1 < t_run:
                    rtile, p2, inp = nxt
                    yr_cur = []
                    for g in range(G):
                        yrg = state.tile([128, GW], f32, name=f"yr_roll{g}",
                                         tag=f"yr{g}")
                        nc.vector.tensor_mul(
                            yrg[:], y_cur[g][:],
                            rtile[:].unsqueeze(1).broadcast_to(
                                [128, NCH // G, BLOC]),
                        )
                        yr_cur.append(yrg)

    t1 = _time.time()
    nc.compile()
    t2 = _time.time()
    if VERBOSE:
        print(f"[build] trace+schedule {t1-t0:.1f}s, bacc compile {t2-t1:.1f}s",
              flush=True)
    return nc


def _host_prep(i, delta_t, W_rec, W_in, b, A, tau, t_run, mm_mode):
    """Shard + lay out the inputs for each core."""
    i = np.asarray(i, dtype=np.float32)
    delta_t = np.asarray(delta_t, dtype=np.float32)
    W_rec = np.asarray(W_rec, dtype=np.float32)
    W_in = np.asarray(W_in, dtype=np.float32)
    b = np.asarray(b, dtype=np.float32)
    A = np.asarray(A, dtype=np.float32)
    tau = np.asarray(tau, dtype=np.float32)

    n_w = {"f32": 1, "bf16": 1, "split6": 3}[mm_mode]

    def tiles_rec(m):   # [512,512] (k, m) -> [128, 16*128]
        return m.reshape(NCH, 128, NCH, 128).transpose(1, 0, 2, 3).reshape(128, -1)

    def tiles_in(m):    # [256,512] (k, m) -> [128, 8*128]
        return m.reshape(FCH, 128, NCH, 128).transpose(1, 0, 2, 3).reshape(128, -1)

    Wt = (W_rec * A[None, :]).T          # Wt[k, m] = A[k] * W_rec[m, k]
    WinT = W_in.T
    if mm_mode == "f32":
        w_arrs = [np.ascontiguousarray(tiles_rec(Wt), dtype=np.float32)]
        win_arrs = [np.ascontiguousarray(tiles_in(WinT), dtype=np.float32)]
    else:
        w_arrs = [tiles_rec(x.astype(np.float32)).astype(x.dtype)
                  for x in _bf16_split(Wt, n_w)]
        win_arrs = [tiles_in(x.astype(np.float32)).astype(x.dtype)
                    for x in _bf16_split(WinT, n_w)]
        w_arrs = [np.ascontiguousarray(x) for x in w_arrs]
        win_arrs = [np.ascontiguousarray(x) for x in win_arrs]

    invtau = np.ascontiguousarray((1.0 / tau).reshape(NCH, 128).T, dtype=np.float32)
    bvec = np.ascontiguousarray(b.reshape(NCH, 128).T, dtype=np.float32)

    in_maps = []
    for c in range(NCORES):
        bsl = slice(c * BLOC, (c + 1) * BLOC)
        ii = i[bsl, :t_run]                    # [16, t, 256]
        def tile_i(x):
            return (x.reshape(BLOC, t_run, FCH, 128)
                    .transpose(3, 1, 2, 0).reshape(128, -1))
        if mm_mode == "f32":
            it_arrs = [np.ascontiguousarray(tile_i(ii), dtype=np.float32)]
        else:
            it_arrs = [np.ascontiguousarray(tile_i(x.astype(np.float32)).astype(x.dtype))
                       for x in _bf16_split(ii, n_w)]
        rt = np.ascontiguousarray(
            (KUNF / np.maximum(delta_t[bsl, :t_run], 1e-30)).T.reshape(1, -1),
            dtype=np.float32)
        m = {"rt": rt, "invtau": invtau, "bvec": bvec}
        for j in range(n_w):
            m[f"wrec{j}"] = w_arrs[j]
            m[f"win{j}"] = win_arrs[j]
            m[f"it{j}"] = it_arrs[j]
        in_maps.append(m)
    return in_maps


def _host_unshard(results, A, t_run):
    """results[c]["yout"]: [t, 128, 64] -> full x [B, T, N] (x = A*y)."""
    A = np.asarray(A, dtype=np.float32)
    out = np.empty((B, t_run, N), dtype=np.float32)
    for c in range(NCORES):
        y = results[c]["yout"].reshape(t_run, 128, NCH, BLOC)
        xc = y.transpose(3, 0, 2, 1).reshape(BLOC, t_run, N)
        out[c * BLOC:(c + 1) * BLOC] = xc * A[None, None, :]
    return out


_BUILD_CACHE = {}


def _get_built(t_run, mm_mode):
    key = (t_run, mm_mode)
    if key not in _BUILD_CACHE:
        _BUILD_CACHE[key] = build(t_run, mm_mode)
    return _BUILD_CACHE[key]


def run(i, delta_t, W_rec, W_in, b, A, tau, t_run=T, mm_mode=MM_DTYPE, **rb_kwargs):
    nc = _get_built(t_run, mm_mode)
    in_maps = _host_prep(i, delta_t, W_rec, W_in, b, A, tau, t_run, mm_mode)
    res = run_bass_kernel_spmd(nc, in_maps, list(range(NCORES)), **rb_kwargs)
    out = _host_unshard(res.results, A, t_run)
    return out, res


def kernel(i, delta_t, W_rec, W_in, b, A, tau):
    out, _ = run(i, delta_t, W_rec, W_in, b, A, tau)
    return out
